# revision 22
# baseline (speedup 1.0000x reference)
"""CBAM3D Trainium2 kernel (8 NeuronCores, SPMD).

Reference computation (per batch sample b):
  avg_pool[c] = mean_{d,h,w} x ; max_pool[c] = max_{d,h,w} x
  ca = sigmoid(relu(avg@w1+b1)@w2+b2) + sigmoid(relu(max@w1+b1)@w2+b2)
  refined = x * ca[c]
  P = [mean_c refined, max_c refined]            # [D,H,W,2]
  sa = sigmoid(conv3d_same(P, conv_w))           # 7x7x7x2 -> 1
  out = refined * sa

Sharding: core i handles sample b=i//2, D-half half=i%2 — 32 own planes,
no raw halo: the d-halo is exchanged as 3 pooled-map planes (bf16,
~50KB) via a pair-wise AllGather after the boundary pairs' phase2a,
with host-provided per-core masks absorbing the lower/upper asymmetry.

Changes vs the 677us baseline (measures ~347us at full DVE clock):
  - input = own 32 planes only (halo pairs eliminated: -6.3MB DMA,
    -3 pair-units of DVE work), 1MB loads leading the DMA queue
    (sband deferred), stage pool triple-buffered (DMA ~320 GB/s)
  - barrier MLP rebuilt PE-side: stats land in [C,2] via two K=1
    matmuls against host [1/N,0]/[0,1] rows, relu bias fused into the
    activation, no DRAM round-trips beyond the collective (~43us -> ~15)
  - conv matmuls emitted per-tap as pooled slots become ready (PE gets
    steady work instead of bursts gated on 4-pair tree groups)
  - spatial-attention apply: ACT pre-expands sa over the channel axis
    so the DVE multiply runs in 2x bf16 mode (was 1x broadcast mode);
    last 4 pairs use the direct 1x mul so the kernel tail is not
    serialized on ACT
"""

from dataclasses import dataclass

import numpy as np
import ml_dtypes

import concourse.bass as bass
import concourse.tile as tile
import concourse.mybir as mybir
from concourse import bacc, bass_isa

F32 = mybir.dt.float32
BF16 = mybir.dt.bfloat16
AX = mybir.AxisListType
OP = mybir.AluOpType
ACT = mybir.ActivationFunctionType


@dataclass(frozen=True)
class Cfg:
    H: int = 64
    W: int = 64
    C: int = 64
    D_LOC: int = 32          # own planes per core
    HID: int = 4             # C // reduction_ratio
    KS: int = 7
    N_CORES: int = 8
    stop_after: str = "full"   # pass1 | mlp | full

    @property
    def HALO(self):
        return self.KS // 2

    @property
    def S(self):
        return self.D_LOC + 2 * self.HALO   # slots in the pooled map

    @property
    def P(self):
        return 2 * self.H                    # partition dim of pair tiles

    @property
    def WP(self):
        return self.W + 2 * self.HALO        # padded pooled-map width

    @property
    def D_TOT(self):
        return 2 * self.D_LOC                # full-sample depth (2 shards)


FULL = Cfg()


def _bc(ap, shape, axis):
    """broadcast ap (by unsqueezing `axis`) to `shape`"""
    return ap.unsqueeze(axis).broadcast_to(shape)


def build_cbam(nc, cfg: Cfg):
    H, W, C = cfg.H, cfg.W, cfg.C
    P, S, WP, HALO = cfg.P, cfg.S, cfg.WP, cfg.HALO
    D_LOC, HID, KS = cfg.D_LOC, cfg.HID, cfg.KS
    PAIRS = D_LOC // 2
    BLK = 4                                  # d-planes per conv block
    NB = D_LOC // BLK
    W2 = W // 2
    NT = KS * KS

    xs = nc.dram_tensor("xs", [D_LOC, H, W, C], F32, kind="ExternalInput").ap()
    w1 = nc.dram_tensor("w1", [C, HID], F32, kind="ExternalInput").ap()
    b1t = nc.dram_tensor("b1t", [HID, 1], F32, kind="ExternalInput").ap()
    w2 = nc.dram_tensor("w2", [HID, C], F32, kind="ExternalInput").ap()
    b2r = nc.dram_tensor("b2r", [2, C], F32, kind="ExternalInput").ap()
    diag2 = nc.dram_tensor("diag2", [2, 2], F32, kind="ExternalInput").ap()
    sband = nc.dram_tensor("sband", [P, NT, H], BF16, kind="ExternalInput").ap()
    mlo = nc.dram_tensor("mlo", [P, 1], F32, kind="ExternalInput").ap()
    mhi = nc.dram_tensor("mhi", [P, 1], F32, kind="ExternalInput").ap()
    out_t = nc.dram_tensor("out", [D_LOC, H, W, C], BF16, kind="ExternalOutput").ap()

    groups = [[i, i + 1] for i in range(0, cfg.N_CORES, 2)]

    with tile.TileContext(nc) as tc:
        with (
            tc.tile_pool(name="consts", bufs=1) as consts,
            tc.tile_pool(name="cache", bufs=1) as cachep,
            tc.tile_pool(name="stage", bufs=3) as stagep,
            tc.tile_pool(name="saexp", bufs=2) as saexpp,
            tc.tile_pool(name="obuf", bufs=2) as obufp,
            tc.tile_pool(name="work", bufs=2) as workp,
            tc.tile_pool(name="dram", bufs=1, space="DRAM") as dram,
            tc.tile_pool(name="ps_small", bufs=1, space="PSUM") as ps_small,
            tc.tile_pool(name="ps_perm", bufs=3, space="PSUM") as ps_perm,
            tc.tile_pool(name="ps_cv", bufs=4, space="PSUM") as ps_cv,
        ):
            # ---------------- constants ----------------
            ones = consts.tile([P, 1], BF16, tag="ones")
            nc.vector.memset(ones, 1.0)
            ones2 = consts.tile([2, 1], F32, tag="ones2")
            nc.vector.memset(ones2, 1.0)

            # bf16 permutation matrices; pooled partition layout is (ci*H+h').
            def diag(t, col_lo, col_hi, base):
                nc.gpsimd.affine_select(
                    out=t[:, col_lo:col_hi], in_=t[:, col_lo:col_hi],
                    compare_op=OP.not_equal, fill=1.0, base=base,
                    pattern=[[-1, col_hi - col_lo]], channel_multiplier=1)

            qa_e = consts.tile([P, P], BF16, tag="qa_e")
            qb_e = consts.tile([P, P], BF16, tag="qb_e")
            qa_o = consts.tile([P, P], BF16, tag="qa_o")
            qb_o = consts.tile([P, P], BF16, tag="qb_o")
            for t in (qa_e, qb_e, qa_o, qb_o):
                nc.gpsimd.memset(t, 0.0)
            diag(qa_e, 0, H, 0)
            diag(qb_e, H, P, 0)
            diag(qa_o, 0, H, -H)
            diag(qb_o, H, P, -H)

            sband_sb = consts.tile([P, NT, H], BF16, tag="sband")
            w1_sb = consts.tile([C, HID], F32, tag="w1")
            w2_sb = consts.tile([HID, C], F32, tag="w2")
            b1t_sb = consts.tile([HID, 1], F32, tag="b1t")
            b2r_sb = consts.tile([2, C], F32, tag="b2r")
            dr0 = consts.tile([1, 2], F32, tag="dr0")
            dr1 = consts.tile([1, 2], F32, tag="dr1")
            mlo_sb = consts.tile([P, 1], F32, tag="mlo")
            mhi_sb = consts.tile([P, 1], F32, tag="mhi")

            # ACT sigmoid table warm + collective-stack warm
            warm_t = workp.tile([1, 1], F32, tag="warm", bufs=1)
            nc.vector.memset(warm_t, 0.0)
            nc.scalar.activation(out=warm_t, in_=warm_t, func=ACT.Sigmoid)
            wu_s = dram.tile([1, 1], F32, tag="wu_s")
            wu_r = dram.tile([2, 1], F32, tag="wu_r")
            nc.sync.dma_start(out=wu_s, in_=warm_t)
            nc.gpsimd.collective_compute(
                "AllGather", OP.bypass, replica_groups=groups,
                ins=[wu_s.opt()], outs=[wu_r.opt()])

            # persistent state
            cache = [cachep.tile([P, W, C], BF16, tag=f"cache{j}",
                                 name=f"cache{j}") for j in range(PAIRS)]
            acc_max = cachep.tile([P, W2, C], BF16, tag="acc_max")
            nc.vector.memset(acc_max, -3.0e38)
            pooled = cachep.tile([P, S, WP], BF16, tag="pooled")
            nc.gpsimd.memset(pooled, 0.0)
            sa_sb = [cachep.tile([H, BLK, W], BF16, tag=f"sa{g}", name=f"sa{g}")
                     for g in range(NB)]
            sa_bf = [cachep.tile([P, BLK // 2, W], BF16, tag=f"sabf{g}",
                                 name=f"sabf{g}") for g in range(NB)]

            # ---------------- pass 1: stream + cast + stats ----------------
            psum_stats = ps_small.tile([1, 8, C], F32, tag="small",
                                       name="stats")
            mm_i = 0
            n_mm = PAIRS * 8
            for j in range(PAIRS):
                for wh in range(2):
                    st = stagep.tile([P, W2, C], F32, tag="stage")
                    deng = nc.sync if (2 * j + wh) % 2 == 0 else nc.scalar
                    deng.dma_start(
                        out=st[:].rearrange("p w c -> p (w c)"),
                        in_=xs[2 * j:2 * j + 2, :, wh * W2:(wh + 1) * W2, :]
                        .rearrange("d h w c -> (d h) (w c)"))
                    nc.scalar.copy(
                        out=cache[j][:, wh * W2:(wh + 1) * W2, :], in_=st)
                    for g in range(4):
                        lo = wh * W2 + g * 8
                        nc.tensor.matmul(
                            out=psum_stats, lhsT=ones[:, :],
                            rhs=cache[j][:, lo:lo + 8, :],
                            start=(mm_i == 0), stop=(mm_i == n_mm - 1))
                        mm_i += 1
                    nc.vector.tensor_tensor(
                        out=acc_max[:].rearrange("p w c -> p (w c)"),
                        in0=acc_max[:].rearrange("p w c -> p (w c)"),
                        in1=cache[j][:, wh * W2:(wh + 1) * W2, :]
                        .rearrange("p w c -> p (w c)"),
                        op=OP.max)

            # const loads deferred here so pass1 x-loads lead the DMA queue
            nc.sync.dma_start(
                out=sband_sb[:].rearrange("p t h -> p (t h)"),
                in_=sband.rearrange("p t h -> p (t h)"))
            nc.scalar.dma_start(out=w1_sb, in_=w1)
            nc.scalar.dma_start(out=w2_sb, in_=w2)
            nc.scalar.dma_start(out=b1t_sb, in_=b1t)
            nc.scalar.dma_start(out=b2r_sb, in_=b2r)
            nc.scalar.dma_start(out=dr0, in_=diag2[0:1, :])
            nc.scalar.dma_start(out=dr1, in_=diag2[1:2, :])
            nc.scalar.dma_start(out=mlo_sb, in_=mlo)
            nc.scalar.dma_start(out=mhi_sb, in_=mhi)

            # ---------------- stats finalize + collective ----------------
            sumc = workp.tile([1, C], F32, tag="sumc", bufs=1)
            nc.vector.tensor_reduce(
                out=sumc, in_=psum_stats[:, :, :].transpose([0, 2, 1]),
                axis=AX.X, op=OP.add)
            # fold acc_max along w (in place): 32 -> 1
            w = W2 // 2
            while w >= 1:
                nc.vector.tensor_tensor(
                    out=acc_max[:, 0:w, :].rearrange("p w c -> p (w c)"),
                    in0=acc_max[:, 0:w, :].rearrange("p w c -> p (w c)"),
                    in1=acc_max[:, w:2 * w, :].rearrange("p w c -> p (w c)"),
                    op=OP.max)
                w //= 2
            maxf = workp.tile([P, C], F32, tag="maxf", bufs=1)
            nc.vector.tensor_copy(out=maxf, in_=acc_max[:, 0, :])
            maxr = workp.tile([P, C], F32, tag="maxr", bufs=1)
            nc.gpsimd.partition_all_reduce(
                out_ap=maxr, in_ap=maxf, channels=P,
                reduce_op=bass_isa.ReduceOp.max)

            snd = dram.tile([2, C], F32, tag="snd")
            rcv = dram.tile([2, 2, C], F32, tag="rcv")
            nc.sync.dma_start(out=snd[0:1, :], in_=sumc)
            nc.sync.dma_start(out=snd[1:2, :], in_=maxr[0:1, :])
            nc.gpsimd.collective_compute(
                "AllGather", OP.bypass, replica_groups=groups,
                ins=[snd.opt()], outs=[rcv.opt()])
            # quad3[p=stat-type, core, c] <- rcv[core, stat-type, c]
            quad3 = workp.tile([1, 4 * C], F32, tag="quad3", bufs=1)
            nc.sync.dma_start(
                out=quad3[:],
                in_=rcv.rearrange("a b c -> (a b c)").unsqueeze(0))
            tot_s = workp.tile([1, C], F32, tag="tot_s", bufs=1)
            nc.vector.tensor_add(out=tot_s, in0=quad3[:, 0:C],
                                 in1=quad3[:, 2 * C:3 * C])
            tot_m = workp.tile([1, C], F32, tag="tot_m", bufs=1)
            nc.vector.tensor_tensor(out=tot_m, in0=quad3[:, C:2 * C],
                                    in1=quad3[:, 3 * C:4 * C], op=OP.max)

            # ---------------- MLP -> ca ----------------
            if cfg.stop_after == "pass1":
                return nc
            # pooled2[C, 2] = tot_s^T @ [1/N, 0] + tot_m^T @ [0, 1]
            psum_p2 = ps_small.tile([C, 2], F32, tag="small", name="p2")
            nc.tensor.matmul(out=psum_p2, lhsT=tot_s, rhs=dr0,
                             start=True, stop=False)
            nc.tensor.matmul(out=psum_p2, lhsT=tot_m, rhs=dr1,
                             start=False, stop=True)
            pooled2 = workp.tile([C, 2], F32, tag="pooled2", bufs=1)
            nc.scalar.copy(out=pooled2, in_=psum_p2)
            psum_h = ps_small.tile([HID, 2], F32, tag="small", name="ph")
            nc.tensor.matmul(out=psum_h, lhsT=w1_sb, rhs=pooled2,
                             start=True, stop=True)
            h2 = workp.tile([HID, 2], F32, tag="h2", bufs=1)
            nc.scalar.activation(out=h2, in_=psum_h, func=ACT.Relu,
                                 bias=b1t_sb[:, :])
            psum_ca = ps_small.tile([2, C], F32, tag="small", name="pca")
            nc.tensor.matmul(out=psum_ca, lhsT=h2, rhs=w2_sb,
                             start=True, stop=True)
            ca_tmp = workp.tile([2, C], F32, tag="ca_tmp", bufs=1)
            nc.vector.tensor_add(out=ca_tmp, in0=psum_ca, in1=b2r_sb)
            ca2 = workp.tile([2, C], F32, tag="ca2", bufs=1)
            nc.scalar.activation(out=ca2, in_=ca_tmp, func=ACT.Sigmoid)
            psum_ca1 = ps_small.tile([1, C], F32, tag="small", name="pca1")
            nc.tensor.matmul(out=psum_ca1, lhsT=ones2, rhs=ca2,
                             start=True, stop=True)
            ca1_bf = workp.tile([1, C], BF16, tag="ca1_bf", bufs=1)
            nc.scalar.copy(out=ca1_bf, in_=psum_ca1)
            ca_bf = consts.tile([P, C], BF16, tag="ca_bf")
            nc.gpsimd.partition_broadcast(out_ap=ca_bf, in_ap=ca1_bf)

            if cfg.stop_after == "mlp":
                return nc

            # ---------------- phase 2 ----------------
            def emit_phase2a(j):
                """refined in-place, channel add/max trees, perm mms."""
                s_e, s_o = HALO + 2 * j, HALO + 2 * j + 1
                nc.vector.tensor_mul(
                    out=cache[j], in0=cache[j],
                    in1=_bc(ca_bf[:, :], [P, W, C], 1))
                rps = {}
                for op in (OP.add, OP.max):
                    rp = workp.tile([P, W], BF16, tag=f"rp{op.name}",
                                    name=f"rp_{j}_{op.name}")
                    t1 = workp.tile([P, W, C // 2], BF16, tag="t1", bufs=2,
                                    name=f"t1_{j}_{op.name}")
                    with nc.allow_low_precision(reason="bf16 pooled sums"):
                        nc.vector.tensor_tensor(
                            out=t1, in0=cache[j][:, :, 0:C // 2],
                            in1=cache[j][:, :, C // 2:], op=op)
                        nc.vector.tensor_tensor(
                            out=t1[:, :, 0:C // 4], in0=t1[:, :, 0:C // 4],
                            in1=t1[:, :, C // 4:], op=op)
                        nc.vector.tensor_reduce(
                            out=rp, in_=t1[:, :, 0:C // 4], axis=AX.X, op=op)
                    rps[op.name] = rp
                pe = ps_perm.tile([P, W], F32, tag="perm", name=f"pe{j}")
                nc.tensor.matmul(out=pe, lhsT=qa_e, rhs=rps["add"],
                                 start=True, stop=False)
                nc.tensor.matmul(out=pe, lhsT=qb_e, rhs=rps["max"],
                                 start=False, stop=True)
                nc.scalar.copy(out=pooled[:, s_e, HALO:HALO + W], in_=pe)
                po = ps_perm.tile([P, W], F32, tag="perm", name=f"po{j}")
                nc.tensor.matmul(out=po, lhsT=qa_o, rhs=rps["add"],
                                 start=True, stop=False)
                nc.tensor.matmul(out=po, lhsT=qb_o, rhs=rps["max"],
                                 start=False, stop=True)
                nc.scalar.copy(out=pooled[:, s_o, HALO:HALO + W], in_=po)

            hsnd = dram.tile([P, 6, W], BF16, tag="hsnd")
            hrcv = dram.tile([2, P, 6, W], BF16, tag="hrcv")

            def emit_exchange():
                """swap 3 boundary pooled planes with the pair neighbor."""
                nc.sync.dma_start(
                    out=hsnd[:, 0:3, :],
                    in_=pooled[:, HALO:HALO + 3, HALO:HALO + W])
                nc.sync.dma_start(
                    out=hsnd[:, 3:6, :],
                    in_=pooled[:, S - HALO - 3:S - HALO, HALO:HALO + W])
                nc.gpsimd.collective_compute(
                    "AllGather", OP.bypass, replica_groups=groups,
                    ins=[hsnd.opt()], outs=[hrcv.opt()])
                hlo = workp.tile([P, 3, W], BF16, tag="hlo", bufs=1)
                nc.sync.dma_start(
                    out=hlo[:],
                    in_=hrcv[0:1, :, 3:6, :].rearrange("a p r w -> (a p) r w"))
                hhi = workp.tile([P, 3, W], BF16, tag="hhi", bufs=1)
                nc.sync.dma_start(
                    out=hhi[:],
                    in_=hrcv[1:2, :, 0:3, :].rearrange("a p r w -> (a p) r w"))
                nc.vector.tensor_scalar_mul(
                    out=pooled[:, 0:HALO, HALO:HALO + W], in0=hlo,
                    scalar1=mlo_sb)
                nc.vector.tensor_scalar_mul(
                    out=pooled[:, S - HALO:S, HALO:HALO + W], in0=hhi,
                    scalar1=mhi_sb)

            def emit_apply(j, g):
                dp = j % 2
                # tail blocks: direct 1x-broadcast mul (the ACT expand would
                # serialize the kernel tail); earlier blocks: ACT pre-expands
                # sa so the DVE multiply runs in 2x bf16 mode
                direct = j >= PAIRS - 4
                for wh in range(2):
                    sto = obufp.tile([P, W2, C], BF16, tag="sto",
                                     name=f"sto{j}_{wh}")
                    if direct:
                        nc.vector.tensor_mul(
                            out=sto, in0=cache[j][:, wh * W2:(wh + 1) * W2, :],
                            in1=_bc(sa_bf[g][:, dp, wh * W2:(wh + 1) * W2],
                                    [P, W2, C], 2))
                    else:
                        sae = saexpp.tile([P, W2, C], BF16, tag="sae",
                                          name=f"sae{j}_{wh}")
                        nc.scalar.copy(
                            out=sae,
                            in_=_bc(sa_bf[g][:, dp, wh * W2:(wh + 1) * W2],
                                    [P, W2, C], 2))
                        nc.vector.tensor_mul(
                            out=sto,
                            in0=cache[j][:, wh * W2:(wh + 1) * W2, :],
                            in1=sae)
                    oeng = nc.sync if (2 * j + wh) % 2 == 0 else nc.scalar
                    oeng.dma_start(
                        out=out_t[2 * j:2 * j + 2, :,
                                  wh * W2:(wh + 1) * W2, :]
                        .rearrange("d h w c -> (d h) (w c)"),
                        in_=sto[:].rearrange("p w c -> p (w c)"))

            # conv emission machinery: events = boundary pairs, exchange,
            # remaining pairs; taps fire as their pooled-slot window fills.
            events = ([("pair", 0), ("pair", 1), ("pair", PAIRS - 2),
                       ("pair", PAIRS - 1), ("exch", None)] +
                      [("pair", j) for j in range(2, PAIRS - 2)])
            slot_ev = {}
            for ev_idx, (kind, j) in enumerate(events):
                if kind == "pair":
                    slot_ev[HALO + 2 * j] = ev_idx
                    slot_ev[HALO + 2 * j + 1] = ev_idx
                else:
                    for s in (0, 1, 2, S - 3, S - 2, S - 1):
                        slot_ev[s] = ev_idx
            from collections import defaultdict
            by_gate = defaultdict(list)
            for g in range(NB):
                for kd in range(KS):
                    lo = g * BLK + kd
                    gate = max(slot_ev[s] for s in range(lo, lo + BLK))
                    by_gate[gate].append((g, kd))

            pcv = {}
            mm_cnt = {g: 0 for g in range(NB)}

            def finish_block(g):
                nc.scalar.activation(out=sa_sb[g], in_=pcv[g],
                                     func=ACT.Sigmoid)
                sa_ev = sa_sb[g].rearrange("h (a b) w -> h a b w", b=2)
                psp = ps_perm.tile([P, BLK // 2, W], F32, tag="perm",
                                   name=f"psp{g}")
                nc.tensor.matmul(out=psp, lhsT=qa_e[0:H, :],
                                 rhs=sa_ev[:, :, 0, :], start=True, stop=False)
                nc.tensor.matmul(out=psp, lhsT=qb_e[0:H, :],
                                 rhs=sa_ev[:, :, 1, :], start=False, stop=True)
                nc.scalar.copy(out=sa_bf[g], in_=psp)
                for j in (2 * g, 2 * g + 1):
                    emit_apply(j, g)

            for ev_idx, (kind, j) in enumerate(events):
                if kind == "pair":
                    emit_phase2a(j)
                else:
                    emit_exchange()
                for (g, kd) in sorted(by_gate.get(ev_idx, [])):
                    if g not in pcv:
                        pcv[g] = ps_cv.tile([H, BLK, W], F32, tag="cv",
                                            name=f"cv{g}")
                    for kw in range(KS):
                        nc.tensor.matmul(
                            out=pcv[g],
                            lhsT=sband_sb[:, kd * KS + kw, :],
                            rhs=pooled[:, g * BLK + kd:g * BLK + kd + BLK,
                                       kw:kw + W],
                            start=(mm_cnt[g] == 0), stop=(mm_cnt[g] == NT - 1),
                            skip_group_check=True)
                        mm_cnt[g] += 1
                    if mm_cnt[g] == NT:
                        finish_block(g)
    return nc


def make_sband(conv_w, cfg: Cfg):
    """Host-side band-matrix construction: [P, KS*KS, H] bf16.

    sband[ci*H+h', kd*KS+kw, h] = conv_w[kd, h'-h+halo, kw, ci] (avg rows
    pre-scaled by 1/C because the pooled map stores channel sums)."""
    H, C, KS, HALO = cfg.H, cfg.C, cfg.KS, cfg.HALO
    cw = np.asarray(conv_w, np.float32)[..., 0]        # [KS,KS,KS,2]
    sb = np.zeros((cfg.P, KS * KS, H), np.float32)
    h = np.arange(H)
    for kd in range(KS):
        for kw in range(KS):
            for ci in range(2):
                scale = (1.0 / C) if ci == 0 else 1.0
                for kh in range(KS):
                    hp = h + kh - HALO                  # h' = h + kh - halo
                    m = (hp >= 0) & (hp < H)
                    sb[ci * H + hp[m], kd * KS + kw, h[m]] = cw[kd, kh, kw, ci] * scale
    return sb.astype(ml_dtypes.bfloat16)


def make_core_inputs(x, w1, b1, w2, b2, sband_np, cfg: Cfg):
    """Shard the full inputs into per-core in_maps."""
    H, W, C, D_LOC = cfg.H, cfg.W, cfg.C, cfg.D_LOC
    x = np.ascontiguousarray(np.asarray(x, np.float32))
    ntot = float(cfg.D_TOT * H * W)
    diag2 = np.array([[1.0 / ntot, 0.0], [0.0, 1.0]], np.float32)
    b1t = np.asarray(b1, np.float32).reshape(cfg.HID, 1)
    b2r = np.repeat(np.asarray(b2, np.float32).reshape(1, C), 2, axis=0)
    b2r = np.ascontiguousarray(b2r)
    in_maps = []
    for core in range(cfg.N_CORES):
        b, half = core // 2, core % 2
        d0 = half * D_LOC
        in_maps.append({
            "xs": np.ascontiguousarray(x[b, d0:d0 + D_LOC]),
            "w1": np.asarray(w1, np.float32).reshape(C, cfg.HID),
            "b1t": np.ascontiguousarray(b1t),
            "w2": np.asarray(w2, np.float32).reshape(cfg.HID, C),
            "b2r": b2r,
            "diag2": diag2,
            "sband": sband_np,
            "mlo": np.full((cfg.P, 1), 1.0 if half == 1 else 0.0, np.float32),
            "mhi": np.full((cfg.P, 1), 1.0 if half == 0 else 0.0, np.float32),
        })
    return in_maps


_COMPILED = {}


def get_compiled(cfg: Cfg = FULL):
    if cfg not in _COMPILED:
        nc = bacc.Bacc("TRN2", target_bir_lowering=False, debug=False,
                       num_devices=cfg.N_CORES)
        build_cbam(nc, cfg)
        nc.compile()
        _COMPILED[cfg] = nc
    return _COMPILED[cfg]


def kernel(x, w1, b1, w2, b2, conv_w):
    from concourse.bass_utils import run_bass_kernel_spmd

    cfg = FULL
    nc = get_compiled(cfg)
    sband_np = make_sband(conv_w, cfg)
    in_maps = make_core_inputs(x, w1, b1, w2, b2, sband_np, cfg)
    res = run_bass_kernel_spmd(nc, in_maps, list(range(cfg.N_CORES)))
    B, D = 4, 64
    out = np.empty((B, D, cfg.H, cfg.W, cfg.C), np.float32)
    for core in range(cfg.N_CORES):
        b, half = core // 2, core % 2
        d0 = half * cfg.D_LOC
        out[b, d0:d0 + cfg.D_LOC] = np.asarray(
            res.results[core]["out"], dtype=np.float32)
    return out
